# revision 1
# baseline (speedup 1.0000x reference)
"""Causal self-attention (B=2, S=2048, E=2048, H=16, rope) on 8 TRN2 NeuronCores.

Sharding: tensor-parallel over heads. Each core owns 2 heads (both batches):
w_qkv rows / w_out columns for its heads; every core reads the full x
(replicated, bf16, pre-transposed) and produces a partial [B*S, E] f32
output; the host sums the 8 partials (the "all-reduce").

Per-core kernel:
  - xT [E, B*S] bf16 serves as matmul rhs (Q/K projections -> QT/KT arrive
    transposed [D, S], the layout attention wants) and as lhsT (V
    projection, natural [S, D]).
  - scores are computed transposed: scoresT[k,q] = KT^T @ QT, in panels of
    512 q columns. exp runs on ScalarE (softmax scale folded into the
    activation scale); causal masking = per-kb column offsets + one bf16
    0/1 mask multiply on the diagonal block; A@V and the sums matmuls
    accumulate only each k-block's causally-valid column range.
  - softmax sums over k (partition dim) use a ones[128,128] matmul that
    produces the column sums already broadcast across all 128 partitions;
    reciprocal + multiply fold normalization into the y^T PSUM evacuation.
  - attn^T feeds A@V as lhsT directly - no transposes anywhere.
  - rope is applied on DVE during QKV-PSUM evacuation with [D, S] cos /
    signed-sin tables; the half-rotation uses a partition-rolled sin table
    so both multiplies are full-width.
"""

import math

import numpy as np
import ml_dtypes

import concourse.bass as bass
import concourse.mybir as mybir
import concourse.tile as tile
from concourse import bacc
from concourse.bass_utils import run_bass_kernel_spmd

B, S, E, H, D = 2, 2048, 2048, 16, 128
NCORES = 8
HL = H // NCORES            # heads per core = 2
NTOK = B * S                # 4096
KE = E // 128               # 16 contraction chunks
NB = S // 128               # 16 k/token blocks per batch
NPANEL = S // 512           # 4 q panels per batch
SOFTMAX_SCALE = 1.0 / math.sqrt(D)
BF16 = mybir.dt.bfloat16
F32 = mybir.dt.float32

ROPE_BASE = 10000.0


def _rope_tables():
    inv_freq = 1.0 / (ROPE_BASE ** (np.arange(0, D, 2, dtype=np.float32) / D))
    pos = np.arange(S, dtype=np.float32)
    freqs = np.outer(pos, inv_freq)               # [S, D/2]
    emb = np.concatenate([freqs, freqs], -1)      # [S, D]
    cosT = np.cos(emb).T.astype(np.float32)       # [D, S]
    sinT = np.sin(emb).T.astype(np.float32)
    sinS = sinT.copy()
    sinS[: D // 2] *= -1.0                        # signed: rotate_half sign folded in
    return np.ascontiguousarray(cosT), np.ascontiguousarray(sinS)


def _attn_panel(nc, pools, b, hl, p, q_sb, k_sb, v_sb, y_sb, mask_sb, ones_kk):
    attnp, psum, evacp = pools
    nkb = 4 * p + 4
    yps = psum.tile([128, 512], F32, tag="yps", bufs=2, name=f"yps{b}{hl}{p}")
    sps = psum.tile([128, 512], F32, tag="sps", bufs=1, name=f"sps{b}{hl}{p}")
    for kb in range(nkb):
        # kb's causally-valid q columns within the panel start at qoff; kb=0
        # always has qoff=0 (start=True initializes all columns), so later
        # kbs may accumulate partial column ranges — no zero-padding needed
        qoff = max(0, kb - 4 * p) * 128
        at = attnp.tile([128, 512], BF16, tag="attn", name=f"at{b}{hl}{p}{kb}")
        ps = psum.tile([128, 512], F32, tag="ps", bufs=3, name=f"sc{b}{hl}{p}{kb}")
        nc.tensor.matmul(
            ps[:, 0:512 - qoff],
            lhsT=k_sb[b][hl][:, kb * 128:(kb + 1) * 128],
            rhs=q_sb[b][hl][:, p * 512 + qoff:(p + 1) * 512],
            start=True,
            stop=True,
        )
        nc.scalar.activation(
            at[:, qoff:512],
            ps[:, 0:512 - qoff],
            mybir.ActivationFunctionType.Exp,
            scale=SOFTMAX_SCALE,
        )
        if kb >= 4 * p:  # diagonal block: zero the k>q half
            nc.vector.tensor_mul(
                at[:, qoff:qoff + 128], at[:, qoff:qoff + 128], mask_sb
            )
        nc.tensor.matmul(
            yps[:, qoff:512],
            lhsT=v_sb[b][:, kb, hl * D:(hl + 1) * D],
            rhs=at[:, qoff:512],
            start=(kb == 0),
            stop=(kb == nkb - 1),
        )
        nc.tensor.matmul(
            sps[:, qoff:512],
            lhsT=ones_kk,
            rhs=at[:, qoff:512],
            start=(kb == 0),
            stop=(kb == nkb - 1),
        )
    rb_sb = evacp.tile([128, 512], F32, tag="rb", name=f"rb{b}{hl}{p}")
    nc.vector.reciprocal_approx_fast(out=rb_sb, in_=sps)
    nc.vector.tensor_mul(y_sb[b][hl][:, p * 512:(p + 1) * 512], yps, rb_sb)


def _emit(nc, tc, xT, wqkvT, w_outT, out, cos_d, sin_d, mask_d):
    from contextlib import ExitStack

    ctx = ExitStack()
    with ctx:
        singles = ctx.enter_context(tc.tile_pool(name="singles", bufs=1))
        xpool = ctx.enter_context(tc.tile_pool(name="xcol", bufs=2))
        persist = ctx.enter_context(tc.tile_pool(name="persist", bufs=1))
        ropet = ctx.enter_context(tc.tile_pool(name="ropet", bufs=3))
        attnp = ctx.enter_context(tc.tile_pool(name="attn", bufs=12))
        evacp = ctx.enter_context(tc.tile_pool(name="evac", bufs=2))
        outp = ctx.enter_context(tc.tile_pool(name="outp", bufs=4))
        psum = ctx.enter_context(tc.tile_pool(name="psum", bufs=2, space="PSUM"))

        # ---- constant tiles (DMAs for non-critical ones deferred below) ----
        wq_sb = [singles.tile([128, 3 * HL * D], BF16, tag=f"wq{ke}", name=f"wq{ke}")
                 for ke in range(KE)]
        wo_sb = singles.tile([128, HL, E], BF16, tag="wo")
        cos_sb = singles.tile([128, S], F32, tag="cos")
        sin_sb = singles.tile([128, S], F32, tag="sin")
        mask_sb = singles.tile([128, 128], BF16, tag="mask")
        ones_kk = singles.tile([128, 128], BF16, tag="oneskk")
        nc.vector.memset(ones_kk, 1.0)
        # bulky constants ride the SWDGE queues so the HWDGE queues carry
        # only the latency-critical wq/xc stream
        nc.gpsimd.dma_start(out=cos_sb, in_=cos_d)
        nc.gpsimd.dma_start(out=sin_sb, in_=sin_d)
        nc.gpsimd.dma_start(out=mask_sb, in_=mask_d)
        for hl in range(HL):
            nc.gpsimd.dma_start(
                out=wo_sb[:, hl, :], in_=w_outT[hl * 128:(hl + 1) * 128, :]
            )

        # ---- persistent per-(b,h) tensors ----
        q_sb = [[persist.tile([128, S], BF16, tag=f"q{b}{h}", name=f"q{b}{h}") for h in range(HL)] for b in range(B)]
        k_sb = [[persist.tile([128, S], BF16, tag=f"k{b}{h}", name=f"k{b}{h}") for h in range(HL)] for b in range(B)]
        v_sb = [persist.tile([128, NB, HL * D], BF16, tag=f"v{b}", name=f"v{b}") for b in range(B)]
        y_sb = [[persist.tile([128, S], BF16, tag=f"y{b}{h}", name=f"y{b}{h}") for h in range(HL)] for b in range(B)]

        pools = (attnp, psum, evacp)

        def proj_batch(b):
            for sb4 in range(S // 512):        # 4 column-blocks of 512 tokens
                tb = b * (S // 512) + sb4
                soff = sb4 * 512
                xc = []
                for ke in range(KE):
                    x1 = xpool.tile([128, 512], BF16, tag=f"xc{ke}", name=f"xc{tb}_{ke}")
                    if tb == 0:
                        # interleave weight/x loads so matmul ke starts after
                        # ~2 small DMAs instead of after the whole input load
                        nc.sync.dma_start(
                            out=wq_sb[ke], in_=wqkvT[ke * 128:(ke + 1) * 128, :]
                        )
                    nc.sync.dma_start(
                        out=x1,
                        in_=xT[ke * 128:(ke + 1) * 128, tb * 512:(tb + 1) * 512],
                    )
                    xc.append(x1)
                # 8 accumulation chains (4 QK rows + 4 V token-blocks) in
                # 3-chain waves, interleaved per-ke: the PE is in-order, so
                # within a wave each arriving xc chunk feeds 3 matmuls back
                # to back instead of one chain stalling on the next DMA
                chains = [("qk", rb) for rb in range(2 * HL)] + [
                    ("v", tsb) for tsb in range(4)
                ]
                if tb == 0:
                    # DMA-paced first block: advance chains in pairs per-ke
                    waves = [chains[i:i + 2] for i in range(0, 8, 2)]
                else:
                    waves = [[c] for c in chains]
                for wv, wave in enumerate(waves):
                    pss = [
                        psum.tile([128, 512], F32, tag="ps", bufs=3,
                                  name=f"p{tb}_{wv}{j}")
                        for j in range(len(wave))
                    ]
                    for ke in range(KE):
                        for j, (kind, idx) in enumerate(wave):
                            if kind == "qk":
                                nc.tensor.matmul(
                                    pss[j],
                                    lhsT=wq_sb[ke][:, idx * 128:(idx + 1) * 128],
                                    rhs=xc[ke],
                                    start=(ke == 0),
                                    stop=(ke == KE - 1),
                                )
                            else:
                                nc.tensor.matmul(
                                    pss[j][:, 0:HL * D],
                                    lhsT=xc[ke][:, idx * 128:(idx + 1) * 128],
                                    rhs=wq_sb[ke][:, 2 * HL * 128:],
                                    start=(ke == 0),
                                    stop=(ke == KE - 1),
                                )
                    for j, (kind, idx) in enumerate(wave):
                        ps = pss[j]
                        if kind == "qk":
                            rb = idx
                            # rope: dst = t*cos + swap(t)*sin_signed, bf16 out
                            dst = (q_sb if rb < HL else k_sb)[b][rb % HL]
                            sl = bass.ds(soff, 512)
                            tsw = ropet.tile([128, 512], F32, tag="tsw", name=f"tsw{tb}{rb}")
                            tco = ropet.tile([128, 512], F32, tag="tco", name=f"tco{tb}{rb}")
                            nc.vector.tensor_mul(tsw[0:64, :], ps[64:128, :], sin_sb[0:64, sl])
                            nc.vector.tensor_mul(tsw[64:128, :], ps[0:64, :], sin_sb[64:128, sl])
                            nc.vector.tensor_mul(tco, ps, cos_sb[:, sl])
                            nc.vector.tensor_add(dst[:, sl], tco, tsw)
                        else:
                            blk = (soff // 128) + idx
                            nc.vector.tensor_copy(v_sb[b][:, blk, :], ps[:, 0:HL * D])

        def outproj_panel(b, p):
            for tkb in range(4 * p, 4 * p + 4):
                tok0 = b * S + tkb * 128
                for oc in range(E // 512):
                    ops = psum.tile([128, 512], F32, tag="ops", bufs=2, name=f"o{b}{tkb}{oc}")
                    for hl in range(HL):
                        nc.tensor.matmul(
                            ops,
                            lhsT=y_sb[b][hl][:, tkb * 128:(tkb + 1) * 128],
                            rhs=wo_sb[:, hl, oc * 512:(oc + 1) * 512],
                            start=(hl == 0),
                            stop=(hl == HL - 1),
                        )
                    ot = outp.tile([128, 512], F32, tag="ot", name=f"ot{b}{tkb}{oc}")
                    if oc % 2 == 0:
                        nc.scalar.copy(ot, ops)
                    else:
                        nc.vector.tensor_copy(ot, ops)
                    nc.sync.dma_start(
                        out=out[tok0:tok0 + 128, oc * 512:(oc + 1) * 512], in_=ot
                    )

        for b in range(B):
            proj_batch(b)
            for p in reversed(range(NPANEL)):
                for hl in range(HL):
                    _attn_panel(nc, pools, b, hl, p, q_sb, k_sb, v_sb, y_sb,
                                mask_sb, ones_kk)
                outproj_panel(b, p)


def build():
    nc = bacc.Bacc("TRN2", target_bir_lowering=False, debug=False)
    xT = nc.dram_tensor("xT", [E, NTOK], BF16, kind="ExternalInput").ap()
    wqkvT = nc.dram_tensor("wqkvT", [E, 3 * HL * D], BF16, kind="ExternalInput").ap()
    w_outT = nc.dram_tensor("w_outT", [HL * D, E], BF16, kind="ExternalInput").ap()
    out = nc.dram_tensor("out", [NTOK, E], F32, kind="ExternalOutput").ap()

    cosT, sinS = _rope_tables()
    cos_d = nc.inline_tensor(cosT, name="cos_t").ap()
    sin_d = nc.inline_tensor(sinS, name="sin_t").ap()
    # maskT01[k, q] = 1 where k <= q (valid), else 0 — transposed-causal
    mask = np.triu(np.ones((128, 128), np.float32)).astype(ml_dtypes.bfloat16)
    mask_d = nc.inline_tensor(mask, name="maskT01").ap()

    with tile.TileContext(nc) as tc:
        _emit(nc, tc, xT, wqkvT, w_outT, out, cos_d, sin_d, mask_d)
    nc.compile()
    return nc


def make_in_maps(x, w_qkv, w_out):
    bf = ml_dtypes.bfloat16
    x2 = np.asarray(x, np.float32).reshape(NTOK, E)
    xT = np.ascontiguousarray(x2.astype(bf).T)                      # [E, NTOK]
    w_qkv = np.asarray(w_qkv, np.float32)
    w_out = np.asarray(w_out, np.float32)
    in_maps = []
    for c in range(NCORES):
        hs = [HL * c + j for j in range(HL)]
        rows = np.concatenate(
            [w_qkv[t * E + h * D:t * E + (h + 1) * D] for t in range(3) for h in hs]
        )                                                           # [768, E]
        wqkvT = np.ascontiguousarray(rows.astype(bf).T)             # [E, 768]
        w_outT = np.ascontiguousarray(
            w_out[:, c * HL * D:(c + 1) * HL * D].astype(bf).T      # [256, E]
        )
        in_maps.append({"xT": xT, "wqkvT": wqkvT, "w_outT": w_outT})
    return in_maps


_NC = None


def kernel(x, w_qkv, w_out):
    global _NC
    if _NC is None:
        _NC = build()
    in_maps = make_in_maps(x, w_qkv, w_out)
    res = run_bass_kernel_spmd(_NC, in_maps, core_ids=list(range(NCORES)))
    total = np.zeros((NTOK, E), np.float32)
    for r in res.results:
        total += r["out"]
    return total.reshape(B, S, E)



# revision 4
# speedup vs baseline: 1.0826x; 1.0826x over previous
"""Causal self-attention (B=2, S=2048, E=2048, H=16, rope) on 8 TRN2 NeuronCores.

Sharding: batch x head-group. Core c owns batch c//4 and heads
4*(c%4)..4*(c%4)+3: w_qkv rows / w_out columns for its heads; each core
reads only its batch's x (bf16, pre-transposed) and produces a partial
[S, E] bf16 output for its batch; the host sums the 4 partials per batch
(the "all-reduce").

Per-core kernel:
  - xT [E, S] bf16 serves as matmul rhs (Q/K projections -> QT/KT arrive
    transposed [D, S], the layout attention wants) and as lhsT (V
    projection, natural [S, D]).
  - scores are computed transposed: scoresT[k,q] = KT^T @ QT, in panels of
    512 q columns. exp runs on ScalarE (softmax scale folded into the
    activation scale); causal masking = per-kb column offsets + one bf16
    0/1 mask multiply on the diagonal block; the A@V matmul accumulates
    only each k-block's causally-valid column range.
  - softmax sums over k: DVE accumulates the exp tiles in f32 across
    k-blocks (partial column ranges follow causality), then a single
    ones[128,128] matmul per panel reduces over the partition dim with the
    result broadcast across all 128 partitions; reciprocal + multiply fold
    normalization into the y^T PSUM evacuation.
  - attn^T feeds A@V as lhsT directly - no transposes anywhere.
  - rope is applied on DVE during QKV-PSUM evacuation with [D, S] cos /
    signed-sin tables; the half-rotation uses a partition-rolled sin table
    so both multiplies are full-width.
  - startup: ~50 throwaway matmuls warm the PE clock (HAM) during the DMA
    init window; constant tables load after the first token-block's
    weight/x stream so they don't starve the critical path.
"""

import math

import numpy as np
import ml_dtypes

import concourse.bass as bass
import concourse.mybir as mybir
import concourse.tile as tile
from concourse import bacc
from concourse.bass_utils import run_bass_kernel_spmd

B, S, E, H, D = 2, 2048, 2048, 16, 128
NCORES = 8
NGRP = 4                    # head groups
HL = H // NGRP              # heads per core = 4
KE = E // 128               # 16 contraction chunks
NB = S // 128               # 16 k/token blocks
NPANEL = S // 512           # 4 q panels
NTB = S // 512              # 4 token blocks for projection
SOFTMAX_SCALE = 1.0 / math.sqrt(D)
BF16 = mybir.dt.bfloat16
F32 = mybir.dt.float32

ROPE_BASE = 10000.0


def _rope_tables():
    inv_freq = 1.0 / (ROPE_BASE ** (np.arange(0, D, 2, dtype=np.float32) / D))
    pos = np.arange(S, dtype=np.float32)
    freqs = np.outer(pos, inv_freq)               # [S, D/2]
    emb = np.concatenate([freqs, freqs], -1)      # [S, D]
    cosT = np.cos(emb).T.astype(np.float32)       # [D, S]
    sinT = np.sin(emb).T.astype(np.float32)
    sinS = sinT.copy()
    sinS[: D // 2] *= -1.0                        # signed: rotate_half sign folded in
    bf = ml_dtypes.bfloat16
    return (np.ascontiguousarray(cosT.astype(bf)),
            np.ascontiguousarray(sinS.astype(bf)))


def _attn_panel(nc, pools, hl, p, q_sb, k_sb, v_sb, y_sb, mask_sb, ones_kk):
    attnp, psum, evacp, accp = pools
    nkb = 4 * p + 4
    yps = psum.tile([128, 512], F32, tag="yps", bufs=2, name=f"yps{hl}{p}")
    acc = accp.tile([128, 512], BF16, tag="acc", bufs=2, name=f"acc{hl}{p}")
    for kb in range(nkb):
        # kb's causally-valid q columns within the panel start at qoff; kb=0
        # always has qoff=0 (start=True / full-width copy initializes all
        # columns), so later kbs may touch partial column ranges only
        qoff = max(0, kb - 4 * p) * 128
        at = attnp.tile([128, 512], BF16, tag="attn", name=f"at{hl}{p}{kb}")
        ps = psum.tile([128, 512], F32, tag="ps", bufs=3, name=f"sc{hl}{p}{kb}")
        nc.tensor.matmul(
            ps[:, 0:512 - qoff],
            lhsT=k_sb[hl][:, kb * 128:(kb + 1) * 128],
            rhs=q_sb[hl][:, p * 512 + qoff:(p + 1) * 512],
            start=True,
            stop=True,
        )
        nc.scalar.activation(
            at[:, qoff:512],
            ps[:, 0:512 - qoff],
            mybir.ActivationFunctionType.Exp,
            scale=SOFTMAX_SCALE,
        )
        if kb >= 4 * p:  # diagonal block: zero the k>q half
            nc.vector.tensor_mul(
                at[:, qoff:qoff + 128], at[:, qoff:qoff + 128], mask_sb
            )
        # softmax denominator: accumulate exp tiles in f32 on DVE (the
        # partition reduction happens once per panel, below)
        if kb == 0:
            nc.vector.tensor_copy(acc, at)
        else:
            nc.vector.tensor_add(
                acc[:, qoff:512], acc[:, qoff:512], at[:, qoff:512]
            )
        nc.tensor.matmul(
            yps[:, qoff:512],
            lhsT=v_sb[:, kb, hl * D:(hl + 1) * D],
            rhs=at[:, qoff:512],
            start=(kb == 0),
            stop=(kb == nkb - 1),
        )
    sps = psum.tile([128, 512], F32, tag="sps", bufs=1, name=f"sps{hl}{p}")
    nc.tensor.matmul(sps, lhsT=ones_kk, rhs=acc, start=True, stop=True)
    rb_sb = evacp.tile([128, 512], F32, tag="rb", name=f"rb{hl}{p}")
    nc.vector.reciprocal_approx_fast(out=rb_sb, in_=sps)
    nc.vector.tensor_mul(y_sb[hl][:, p * 512:(p + 1) * 512], yps, rb_sb)


def _emit(nc, tc, xT, wqkvT, w_outT, out, cos_d, sin_d, mask_d):
    from contextlib import ExitStack

    ctx = ExitStack()
    with ctx:
        singles = ctx.enter_context(tc.tile_pool(name="singles", bufs=1))
        xpool = ctx.enter_context(tc.tile_pool(name="xcol", bufs=2))
        persist = ctx.enter_context(tc.tile_pool(name="persist", bufs=1))
        ropet = ctx.enter_context(tc.tile_pool(name="ropet", bufs=2))
        attnp = ctx.enter_context(tc.tile_pool(name="attn", bufs=6))
        evacp = ctx.enter_context(tc.tile_pool(name="evac", bufs=2))
        accp = ctx.enter_context(tc.tile_pool(name="accp", bufs=2))
        outp = ctx.enter_context(tc.tile_pool(name="outp", bufs=4))
        psum = ctx.enter_context(tc.tile_pool(name="psum", bufs=2, space="PSUM"))

        # ---- constant tiles ----
        wq_sb = [singles.tile([128, 3 * HL * D], BF16, tag=f"wq{ke}", name=f"wq{ke}")
                 for ke in range(KE)]
        wo_sb = singles.tile([128, HL, E], BF16, tag="wo")
        cos_sb = singles.tile([128, S], BF16, tag="cos")
        sin_sb = singles.tile([128, S], BF16, tag="sin")
        mask_sb = singles.tile([128, 128], BF16, tag="mask")
        ones_kk = singles.tile([128, 128], BF16, tag="oneskk")
        nc.vector.memset(ones_kk, 1.0)

        # ---- PE warm-up: keep the HAM activity window busy during the DMA
        # init dead time so real matmuls start at full clock ----
        warm = psum.tile([128, 512], F32, tag="sps", bufs=1, name="warm")
        for _ in range(48):
            nc.tensor.matmul(warm[:, 0:128], lhsT=ones_kk, rhs=ones_kk,
                             start=True, stop=True)

        # ---- persistent per-head tensors ----
        q_sb = [persist.tile([128, S], BF16, tag=f"q{h}", name=f"q{h}") for h in range(HL)]
        k_sb = [persist.tile([128, S], BF16, tag=f"k{h}", name=f"k{h}") for h in range(HL)]
        v_sb = persist.tile([128, NB, HL * D], BF16, tag="v", name="v")
        y_sb = [persist.tile([128, S], BF16, tag=f"y{h}", name=f"y{h}") for h in range(HL)]

        pools = (attnp, psum, evacp, accp)

        def load_consts(stage):
            # bulky constants ride the same ordered HW queue, but *after*
            # the critical first-block weight/x stream
            if stage == 0:
                nc.sync.dma_start(out=cos_sb, in_=cos_d)
                nc.sync.dma_start(out=sin_sb, in_=sin_d)
            else:
                nc.sync.dma_start(out=mask_sb, in_=mask_d)
                for hl in range(HL):
                    nc.sync.dma_start(
                        out=wo_sb[:, hl, :], in_=w_outT[hl * 128:(hl + 1) * 128, :]
                    )

        def proj_block(tb):
            soff = tb * 512
            xc = []
            for ke in range(KE):
                x1 = xpool.tile([128, 512], BF16, tag=f"xc{ke}", name=f"xc{tb}_{ke}")
                if tb == 0:
                    # interleave weight/x loads so matmul ke starts after
                    # ~2 small DMAs instead of after the whole input load
                    nc.sync.dma_start(
                        out=wq_sb[ke], in_=wqkvT[ke * 128:(ke + 1) * 128, :]
                    )
                nc.sync.dma_start(
                    out=x1,
                    in_=xT[ke * 128:(ke + 1) * 128, tb * 512:(tb + 1) * 512],
                )
                xc.append(x1)
            if tb == 0:
                load_consts(0)
            elif tb == 1:
                load_consts(1)
            # 12 accumulation chains (8 QK rows + 4 V token-blocks): the PE
            # is in-order, so within a wave each arriving xc chunk feeds the
            # wave's matmuls back to back instead of one chain stalling on
            # the next DMA
            chains = [("qk", rb) for rb in range(2 * HL)] + [
                ("v", tsb) for tsb in range(4)
            ]
            if tb == 0:
                # DMA-paced first block: advance chains in pairs per-ke
                waves = [chains[i:i + 2] for i in range(0, len(chains), 2)]
            else:
                waves = [[c] for c in chains]
            for wv, wave in enumerate(waves):
                pss = [
                    psum.tile([128, 512], F32, tag="ps", bufs=3,
                              name=f"p{tb}_{wv}{j}")
                    for j in range(len(wave))
                ]
                for ke in range(KE):
                    for j, (kind, idx) in enumerate(wave):
                        if kind == "qk":
                            nc.tensor.matmul(
                                pss[j],
                                lhsT=wq_sb[ke][:, idx * 128:(idx + 1) * 128],
                                rhs=xc[ke],
                                start=(ke == 0),
                                stop=(ke == KE - 1),
                            )
                        else:
                            nc.tensor.matmul(
                                pss[j],
                                lhsT=xc[ke][:, idx * 128:(idx + 1) * 128],
                                rhs=wq_sb[ke][:, 2 * HL * 128:],
                                start=(ke == 0),
                                stop=(ke == KE - 1),
                            )
                for j, (kind, idx) in enumerate(wave):
                    ps = pss[j]
                    if kind == "qk":
                        rb = idx
                        # rope: dst = t*cos + swap(t)*sin_signed, bf16 out
                        dst = (q_sb if rb < HL else k_sb)[rb % HL]
                        sl = bass.ds(soff, 512)
                        tsw = ropet.tile([128, 512], F32, tag="tsw", name=f"tsw{tb}{rb}")
                        tco = ropet.tile([128, 512], F32, tag="tco", name=f"tco{tb}{rb}")
                        nc.vector.tensor_mul(tsw[0:64, :], ps[64:128, :], sin_sb[0:64, sl])
                        nc.vector.tensor_mul(tsw[64:128, :], ps[0:64, :], sin_sb[64:128, sl])
                        nc.vector.tensor_mul(tco, ps, cos_sb[:, sl])
                        nc.vector.tensor_add(dst[:, sl], tco, tsw)
                    else:
                        blk = (soff // 128) + idx
                        nc.vector.tensor_copy(v_sb[:, blk, :], ps)

        def outproj_panel(p):
            for tkb in range(4 * p, 4 * p + 4):
                tok0 = tkb * 128
                for oc in range(E // 512):
                    ops = psum.tile([128, 512], F32, tag="ops", bufs=2, name=f"o{tkb}{oc}")
                    for hl in range(HL):
                        nc.tensor.matmul(
                            ops,
                            lhsT=y_sb[hl][:, tkb * 128:(tkb + 1) * 128],
                            rhs=wo_sb[:, hl, oc * 512:(oc + 1) * 512],
                            start=(hl == 0),
                            stop=(hl == HL - 1),
                        )
                    ot = outp.tile([128, 512], BF16, tag="ot", name=f"ot{tkb}{oc}")
                    if oc % 2 == 0:
                        nc.scalar.copy(ot, ops)
                    else:
                        nc.vector.tensor_copy(ot, ops)
                    nc.sync.dma_start(
                        out=out[tok0:tok0 + 128, oc * 512:(oc + 1) * 512], in_=ot
                    )

        for tb in range(NTB):
            proj_block(tb)
        for p in reversed(range(NPANEL)):
            for hl in range(HL):
                _attn_panel(nc, pools, hl, p, q_sb, k_sb, v_sb, y_sb,
                            mask_sb, ones_kk)
            outproj_panel(p)


def build():
    nc = bacc.Bacc("TRN2", target_bir_lowering=False, debug=False)
    xT = nc.dram_tensor("xT", [E, S], BF16, kind="ExternalInput").ap()
    wqkvT = nc.dram_tensor("wqkvT", [E, 3 * HL * D], BF16, kind="ExternalInput").ap()
    w_outT = nc.dram_tensor("w_outT", [HL * D, E], BF16, kind="ExternalInput").ap()
    out = nc.dram_tensor("out", [S, E], BF16, kind="ExternalOutput").ap()

    cosT, sinS = _rope_tables()
    cos_d = nc.inline_tensor(cosT, name="cos_t").ap()
    sin_d = nc.inline_tensor(sinS, name="sin_t").ap()
    # maskT01[k, q] = 1 where k <= q (valid), else 0 — transposed-causal
    mask = np.triu(np.ones((128, 128), np.float32)).astype(ml_dtypes.bfloat16)
    mask_d = nc.inline_tensor(mask, name="maskT01").ap()

    with tile.TileContext(nc) as tc:
        _emit(nc, tc, xT, wqkvT, w_outT, out, cos_d, sin_d, mask_d)
    nc.compile()
    return nc


def core_shard(c):
    """core c -> (batch, head list)."""
    b, g = c // NGRP, c % NGRP
    return b, [HL * g + j for j in range(HL)]


def make_in_maps(x, w_qkv, w_out):
    bf = ml_dtypes.bfloat16
    x2 = np.asarray(x, np.float32).reshape(B, S, E)
    xTs = [np.ascontiguousarray(x2[b].astype(bf).T) for b in range(B)]  # [E, S]
    w_qkv = np.asarray(w_qkv, np.float32)
    w_out = np.asarray(w_out, np.float32)
    in_maps = []
    for c in range(NCORES):
        b, hs = core_shard(c)
        rows = np.concatenate(
            [w_qkv[t * E + h * D:t * E + (h + 1) * D] for t in range(3) for h in hs]
        )                                                           # [1536, E]
        wqkvT = np.ascontiguousarray(rows.astype(bf).T)             # [E, 1536]
        cols = np.concatenate([w_out[:, h * D:(h + 1) * D] for h in hs], axis=1)
        w_outT = np.ascontiguousarray(cols.astype(bf).T)            # [512, E]
        in_maps.append({"xT": xTs[b], "wqkvT": wqkvT, "w_outT": w_outT})
    return in_maps


def gather(results):
    total = np.zeros((B, S, E), np.float32)
    for c, r in enumerate(results):
        b, _ = core_shard(c)
        total[b] += np.asarray(r["out"], np.float32)
    return total


_NC = None


def kernel(x, w_qkv, w_out):
    global _NC
    if _NC is None:
        _NC = build()
    in_maps = make_in_maps(x, w_qkv, w_out)
    res = run_bass_kernel_spmd(_NC, in_maps, core_ids=list(range(NCORES)))
    return gather([r for r in res.results])
